# revision 36
# baseline (speedup 1.0000x reference)
"""Trainium2 Bass kernel for nn_MLPFusionLoRA (MoE-routed fused MLP + LoRA).

Sharding: the 32 (sample, 256-token-tile) CHUNKS are load-balanced across the
8 NeuronCores by their active-expert count: chunks are sorted by nact
descending and dealt round-robin, so core c's slot s holds a chunk with
nact <= pattern[s], where pattern[s] = max over cores at that slot. One SPMD
program compiled per pattern: slot s runs pattern[s] LoRA/gate/combine slots
(padded slots get zero weights -> exact zeros). This cuts the per-core LoRA
slot count from ntt*max(nact) to sum(pattern) — for skewed masks a ~10%
PE-slot reduction on the non-fc work.

Each chunk's LoRA/gate weights (a1/a2/b1/b2/gb/rsm, ~2MB packed) stream on
the idle gpsimd DMA queue one tile ahead, double-buffered; fc1/fc2 weights
stay resident. The masked routing combine mixes modalities at fixed sample,
so every chunk is self-contained.

Kernel structure (per chunk; feature-major, tokens on the matmul free dim):
  x1_i  = fc1_w @ x_i^T                                  [3072, T]
  t_k   = a1_w[act k] @ x_k^T  (rank 16 in 32-slot)      [128, T]
  w[i,t,e] = E_ie / (D_i + 4e-6) * mask[e]*mask[i],  E = exp(gate logits)
  x1_k += B1^T.T @ (t * wexp_k)     <- routing combine folded into one matmul
  h_i   = gelu(x1_i + fc1_b)
  y_i   = fc2_w @ h_i + B2^T.T @ (u * wexp_k) + fc2_b,  u_k = a2_w[act k] @ h_k

Scheduling notes (all measured from NTFF profiles):
- matmul issue rate is ~109ns per 256-col slot, back-to-back; wall time ==
  slot count, so every optimization is a slot-count or idle-gap reduction.
- t1/u clusters col-tile (tile_position=(0,32k)) so up to A 32-col matmuls
  stream concurrently; dz/wexp routing matmuls row-tile ((32k,0)) the same
  way. f32r cannot col-tile (ISA reject); row tiling verified exact.
- interleaving small tiled matmuls into the full-width stream costs ~250ns
  of weight-load serialization per island, so the LoRA combine matmuls stay
  full-width (their streaming slot is the floor anyway).
- routing chain is split (a / b1 / b2) across fc2 interleave points so the
  PE never waits on the chain's serialized DVE stages.
- x / y DRAM tensors are tile-major so every DMA is fully contiguous; w1 is
  j-major so fc1 can start as soon as the first chunks land; per-dma_start
  cost ~0.6us of issuing-engine time, so x is ONE fused DMA per tile.
- PE warm-up spam during the ~10us DMA preamble keeps the HAM clock hot.
- Output stored bf16 (halves out-DMA; rel-err impact ~1e-3).
"""

from contextlib import ExitStack

import numpy as np
import ml_dtypes

import concourse.bacc as bacc
import concourse.mybir as mybir
import concourse.tile as tile
from concourse import bass_utils

F32 = mybir.dt.float32
F32R = mybir.dt.float32r
BF16 = mybir.dt.bfloat16
NPBF = ml_dtypes.bfloat16

M, B, NT, C, H = 4, 8, 1024, 768, 3072
CK, HK = C // 128, H // 128  # 6, 24
T = 256                      # token tile
NTT = NT // T                # 4
RSW = 224 + 128 * 4          # rsm width
AF = mybir.ActivationFunctionType
ALU = mybir.AluOpType

_CACHE = {}


def _build_program(pattern, gelu=AF.Gelu):
    """pattern[s] = number of active (modality==expert) slots at tile s."""
    ntt = len(pattern)
    nc = bacc.Bacc("TRN2", target_bir_lowering=False, debug=False)

    dp = lambda name, shape, dt: nc.dram_tensor(name, shape, dt, kind="ExternalInput").ap()
    xt = dp("xt", [ntt, M, 128, CK * T], BF16)     # xt[s,m,p,c*T+t]
    w1 = dp("w1", [HK, 128, CK * 128], BF16)       # j-major
    w2 = dp("w2", [HK, 128, C], BF16)
    a1 = dp("a1", [ntt, 128, CK * 128], BF16)      # per-slot LoRA/gate packs
    a2 = dp("a2", [ntt, 128, HK * 128], BF16)
    b1 = dp("b1", [ntt, 128, H], BF16)
    b2 = dp("b2", [ntt, 128, C], BF16)
    gb = dp("gb", [ntt, 128, 1], F32)
    rsm = dp("rsm", [ntt, 128, RSW], F32R)
    f1b = dp("f1b", [128, HK], F32)
    f2b = dp("f2b", [128, CK], F32)
    yt = nc.dram_tensor("yt", [M, ntt, 2, 128, 3 * T], BF16, kind="ExternalOutput").ap()

    with tile.TileContext(nc) as tc, ExitStack() as ctx:
        wp = ctx.enter_context(tc.tile_pool(name="wts", bufs=1))
        lw = ctx.enter_context(tc.tile_pool(name="lw", bufs=2))
        xp = ctx.enter_context(tc.tile_pool(name="xin", bufs=2))
        hp = ctx.enter_context(tc.tile_pool(name="hts", bufs=4))
        sp = ctx.enter_context(tc.tile_pool(name="smal", bufs=2))
        wx = ctx.enter_context(tc.tile_pool(name="wexp", bufs=3))
        syp = ctx.enter_context(tc.tile_pool(name="yout", bufs=2))
        ssp = ctx.enter_context(tc.tile_pool(name="sS", bufs=3))
        pmm = ctx.enter_context(tc.tile_pool(name="pmm", bufs=4, space="PSUM"))
        ptu = ctx.enter_context(tc.tile_pool(name="ptu", bufs=2, space="PSUM"))
        prt = ctx.enter_context(tc.tile_pool(name="prt", bufs=2, space="PSUM"))

        # ---- resident (shared) weights ----
        w1s = wp.tile([128, CK * H], BF16)
        w2s = wp.tile([128, HK * C], BF16)
        f1bs = wp.tile([128, HK], F32)
        f2bs = wp.tile([128, CK], F32)
        gateb = wp.tile([1, 64], BF16)

        # slot-0 a1 head first on sync: it feeds the PE warm-up below
        a1s0 = lw.tile([128, CK * 128], BF16, tag="a1", name="a1s_0")
        nc.sync.dma_start(a1s0[:, 0:384], a1[0, :, 0:384])

        warm = prt.tile([128, T], F32, tag="rt", bufs=2, name="warm")
        for w in range(14):
            nc.tensor.matmul(warm[:], a1s0[:, 0:128], a1s0[:, 128:128 + T],
                             start=True, stop=True)

        def load_x(s):
            A = pattern[s]
            xs = xp.tile([128, M * CK * T], BF16, tag="xs", name=f"xs_{s}")
            # tile 0: x rides the preamble-idle scalar+gpsimd queues (half
            # each, in parallel) so the sync queue streams a1+w1 instead of
            # serializing the 1.5MB x transfer ahead of every weight chunk
            if s == 0:
                hw = M // 2 * CK * T
                nc.scalar.dma_start(
                    xs[:, 0:hw].rearrange("p (m f) -> p m f", m=M // 2),
                    xt[s, 0:M // 2].rearrange("m p f -> p m f"))
                nc.gpsimd.dma_start(
                    xs[:, hw:].rearrange("p (m f) -> p m f", m=M // 2),
                    xt[s, M // 2:].rearrange("m p f -> p m f"))
            else:
                nc.sync.dma_start(xs[:].rearrange("p (m f) -> p m f", m=M),
                                  xt[s].rearrange("m p f -> p m f"))
            st = {"tt": s, "A": A, "xs": xs,
                  "fco": list(range(A, M)) + list(range(A)),
                  "xv": lambda m, c, _x=xs: _x[:, (m * CK + c) * T:(m * CK + c + 1) * T]}
            if s == 0:
                a1s = a1s0
                nc.sync.dma_start(a1s[:, 384:], a1[0, :, 384:])
                nc.sync.dma_start(f1bs[:], f1b[:])
                # w1 j0..11 on sync, j12..23 on gpsimd (behind its x half):
                # arrival order then matches fc1's j-ascending consumption
                for j in range(0, HK // 2):
                    nc.sync.dma_start(w1s[:, j * CK * 128:(j + 1) * CK * 128], w1[j])
                b1s = lw.tile([128, H], BF16, tag="b1", name="b1s_0")
                nc.sync.dma_start(b1s[:], b1[0])
                # gpsimd bulk gated behind the x DMA via a tiny SBUF copy
                nc.gpsimd.dma_start(gateb[:], xs[0:1, (M - 1) * CK * T:(M - 1) * CK * T + 64])
                gbs = lw.tile([128, 1], F32, tag="gb", name="gb_0")
                nc.gpsimd.dma_start(gbs[:], gb[0])
                rsml = lw.tile([128, RSW], F32R, tag="rsm", name="rsm_0")
                nc.gpsimd.dma_start(rsml[:], rsm[0])
                for j in range(HK // 2, HK):
                    nc.gpsimd.dma_start(w1s[:, j * CK * 128:(j + 1) * CK * 128], w1[j])
                nc.gpsimd.dma_start(f2bs[:], f2b[:])
                a2s = lw.tile([128, HK * 128], BF16, tag="a2", name="a2s_0")
                nc.gpsimd.dma_start(a2s[:], a2[0])
                b2s = lw.tile([128, C], BF16, tag="b2", name="b2s_0")
                nc.gpsimd.dma_start(b2s[:], b2[0])
                for k in range(HK):
                    nc.gpsimd.dma_start(w2s[:, k * C:(k + 1) * C], w2[k])
            else:
                # next chunk's LoRA/gate set rides the idle gpsimd queue,
                # double-buffered; lands mid-previous-tile, used next tile
                rsml = lw.tile([128, RSW], F32R, tag="rsm", name=f"rsm_{s}")
                nc.gpsimd.dma_start(rsml[:], rsm[s])
                gbs = lw.tile([128, 1], F32, tag="gb", name=f"gb_{s}")
                nc.gpsimd.dma_start(gbs[:], gb[s])
                a1s = lw.tile([128, CK * 128], BF16, tag="a1", name=f"a1s_{s}")
                nc.gpsimd.dma_start(a1s[:], a1[s])
                b1s = lw.tile([128, H], BF16, tag="b1", name=f"b1s_{s}")
                nc.gpsimd.dma_start(b1s[:], b1[s])
                a2s = lw.tile([128, HK * 128], BF16, tag="a2", name=f"a2s_{s}")
                nc.gpsimd.dma_start(a2s[:], a2[s])
                b2s = lw.tile([128, C], BF16, tag="b2", name=f"b2s_{s}")
                nc.gpsimd.dma_start(b2s[:], b2[s])
            st.update(a1s=a1s, a2s=a2s, b1s=b1s, b2s=b2s, gbs=gbs, rsml=rsml)
            return st

        def emit_t1(st):
            # gate logits ride inside the a1 matmul: block k of a1 carries
            # gate_w[act k] in cols 32k+0..3 and a1_w[act k] ranks in cols
            # 32k+4..19. c-outer / k-inner: concurrent col groups.
            tt, xv, A = st["tt"], st["xv"], st["A"]
            if A == 0:
                return
            a1s = st["a1s"]
            t1 = ptu.tile([128, T], F32, tag="tu", name=f"t1_{tt}")
            for c in range(CK):
                for k in range(A):
                    nc.tensor.matmul(t1[32 * k:32 * k + 32, :],
                                     a1s[:, c * 128 + 32 * k: c * 128 + 32 * k + 32],
                                     xv(k, c), start=(c == 0), stop=(c == CK - 1),
                                     tile_position=(0, 32 * k))
            st["t1"] = t1

        def emit_chains_a(st):
            # slot-k chain lives at partition base 32k so dz and wexp run as
            # row-tiled concurrent groups. rb outputs land at rows 32k via
            # stationary column placement, keeping DVE ops single-base.
            tt, A = st["tt"], st["A"]
            if A == 0:
                return
            gbs, rsml = st["gbs"], st["rsml"]
            t1s = sp.tile([128, T], F32, tag="t1s", name=f"t1s_{tt}")
            nc.vector.tensor_copy(t1s[:], st["t1"][:])
            st["t1s"] = t1s
            EiAll = sp.tile([32 * (A - 1) + 4, T], F32R, tag="Ei", bufs=2,
                            name=f"Ei_{tt}")
            for k in range(A):
                nc.scalar.activation(EiAll[32 * k:32 * k + 4, :],
                                     st["t1"][32 * k:32 * k + 4, :],
                                     AF.Exp, bias=gbs[32 * k:32 * k + 4, 0:1])
            # den = sum_e mask_e*E_e + eps; the reference's +1e-6*sum(E)
            # regularizer is a constant 4e-6 here (as if E=1): relative
            # effect ~1e-6 when any expert is active, exact zero otherwise.
            dzs, dens = [], []
            for k in range(A):
                dz = prt.tile([1, T], F32, tag="rt", bufs=2, name=f"dz_{tt}_{k}")
                nc.tensor.matmul(dz[:], rsml[32 * k:32 * k + 4, k:k + 1],
                                 EiAll[32 * k:32 * k + 4, :], start=True, stop=True,
                                 tile_position=(32 * k, 0))
                dzs.append(dz)
            for k in range(A):
                den = sp.tile([1, T], F32, tag="den", bufs=4, name=f"den_{tt}_{k}")
                nc.vector.tensor_scalar_add(den[:], dzs[k][:], 4e-6)
                dens.append(den)
            st["Ei"], st["dens"] = EiAll, dens

        def emit_chains_b1(st):
            tt, A = st["tt"], st["A"]
            if A == 0:
                return
            rsml = st["rsml"]
            EiAll = st["Ei"]
            rvs = []
            for k in range(A):
                rv32 = sp.tile([1, T], F32, tag="rv32", name=f"rv32_{tt}_{k}")
                nc.vector.reciprocal_approx_fast(rv32[:], st["dens"][k][:])
                rv = sp.tile([1, T], F32R, tag="rv", bufs=4, name=f"rv_{tt}_{k}")
                nc.vector.tensor_copy(rv[:], rv32[:])
                rvs.append(rv)
            rbs = []
            for k in range(A):
                rb = prt.tile([32 * k + 4, T], F32, tag="rt", bufs=2,
                              name=f"rb_{tt}_{k}")
                nc.tensor.matmul(rb[:], rsml[0:1, 8 + 128 * k:8 + 128 * k + 32 * k + 4],
                                 rvs[k][:], start=True, stop=True)
                rbs.append(rb)
            wfAll = sp.tile([32 * (A - 1) + 4, T], F32R, tag="wf", bufs=2,
                            name=f"wf_{tt}")
            for k in range(A):
                nc.vector.tensor_tensor(wfAll[32 * k:32 * k + 4, :],
                                        EiAll[32 * k:32 * k + 4, :],
                                        rbs[k][32 * k:32 * k + 4, :], ALU.mult)
            st["wf"] = wfAll

        def emit_chains_b2(st):
            tt, A = st["tt"], st["A"]
            if A == 0:
                return
            rsml = st["rsml"]
            wfAll = st["wf"]
            wexps, wexp_pss = [], []
            for k in range(A):
                wexp_ps = prt.tile([128, T], F32, tag="rt", bufs=2,
                                   name=f"wexp_ps_{tt}_{k}")
                nc.tensor.matmul(wexp_ps[:],
                                 rsml[32 * k:32 * k + 4, 224 + k * 128:224 + (k + 1) * 128],
                                 wfAll[32 * k:32 * k + 4, :], start=True, stop=True,
                                 tile_position=(32 * k, 0))
                wexp_pss.append(wexp_ps)
            for k in range(A):
                wexp = wx.tile([128, T], F32, tag="wexp", name=f"wexp_{tt}_{k}")
                nc.vector.tensor_copy(wexp[:], wexp_pss[k][:])
                wexps.append(wexp)
            st["wexps"] = wexps

        def emit_S(st):
            tt, A = st["tt"], st["A"]
            Ss = []
            for k in range(A):
                S = ssp.tile([128, T], BF16, tag="S1", name=f"S_{tt}_{k}")
                nc.vector.tensor_tensor(S[:], st["t1s"][:], st["wexps"][k][:], ALU.mult)
                Ss.append(S)
            st["Ss"] = Ss

        def emit_fc1(st, i_list, j_range=None):
            tt, xv, A = st["tt"], st["xv"], st["A"]
            b1s = st["b1s"]
            hs = st.setdefault("hs", {})
            for i in i_list:
                if i in hs:
                    hsi = hs[i]
                else:
                    hsi = hp.tile([128, HK * T], BF16, tag="hs", name=f"hs_{tt}_{i}")
                    hs[i] = hsi
                lora = i < A
                for j in (range(HK) if j_range is None else j_range):
                    x1 = pmm.tile([128, T], F32, tag="mm", name=f"x1_{tt}_{i}_{j}")
                    for c in range(CK):
                        nc.tensor.matmul(x1[:], w1s[:, (j * CK + c) * 128:(j * CK + c + 1) * 128],
                                         xv(i, c), start=(c == 0),
                                         stop=(not lora and c == CK - 1))
                    if lora:
                        nc.tensor.matmul(x1[:], b1s[:, 128 * j:128 * (j + 1)], st["Ss"][i][:],
                                         start=False, stop=True)
                    nc.scalar.activation(hsi[:, j * T:(j + 1) * T], x1[:], gelu,
                                         bias=f1bs[:, j:j + 1])

        def emit_u(st):
            tt, A = st["tt"], st["A"]
            if A == 0:
                return
            a2s = st["a2s"]
            u = ptu.tile([128, T], F32, tag="tu", name=f"u_{tt}")
            hs = st["hs"]
            for j in range(HK):
                for k in range(A):
                    nc.tensor.matmul(u[32 * k:32 * k + 32, :],
                                     a2s[:, j * 128 + 32 * k: j * 128 + 32 * k + 32],
                                     hs[k][:, j * T:(j + 1) * T],
                                     start=(j == 0), stop=(j == HK - 1),
                                     tile_position=(0, 32 * k))
            st["u"] = u

        def emit_us_S2(st):
            tt, A = st["tt"], st["A"]
            if A == 0:
                return
            us = sp.tile([128, T], F32, tag="us", name=f"us_{tt}")
            nc.vector.tensor_copy(us[:], st["u"][:])
            S2s = []
            for k in range(A):
                S2 = ssp.tile([128, T], BF16, tag="S2", name=f"S2_{tt}_{k}")
                nc.vector.tensor_tensor(S2[:], us[:], st["wexps"][k][:], ALU.mult)
                S2s.append(S2)
            st["S2s"] = S2s

        def emit_fc2(st, i_list):
            tt, A = st["tt"], st["A"]
            b2s = st["b2s"]
            for i in i_list:
                lora = i < A
                for half in range(2):
                    # 3 c-blocks per output DMA: per-dma_start engine cost
                    # (~0.6us) dominates the writeback tail otherwise
                    ysb = syp.tile([128, 3 * T], BF16, tag="y", name=f"ysb_{tt}_{i}_{half}")
                    for jj in range(3):
                        j = half * 3 + jj
                        y = pmm.tile([128, T], F32, tag="mm", name=f"y_{tt}_{i}_{j}")
                        for k in range(HK):
                            nc.tensor.matmul(y[:], w2s[:, k * C + 128 * j: k * C + 128 * (j + 1)],
                                             st["hs"][i][:, k * T:(k + 1) * T],
                                             start=(k == 0),
                                             stop=(not lora and k == HK - 1))
                        if lora:
                            nc.tensor.matmul(y[:], b2s[:, 128 * j:128 * (j + 1)], st["S2s"][i][:],
                                             start=False, stop=True)
                        nc.vector.tensor_scalar_add(ysb[:, jj * T:(jj + 1) * T], y[:],
                                                    f2bs[:, j:j + 1])
                    qe = (nc.sync, nc.scalar)[(i + half) % 2]
                    qe.dma_start(yt[i, tt, half], ysb[:])

        def warm2(n):
            for w in range(n):
                nc.tensor.matmul(warm[:], a1s0[:, 0:128], a1s0[:, 128:128 + T],
                                 start=True, stop=True)

        st = load_x(0)
        emit_t1(st)
        # tile 0: chains_a first so its Exp work runs on the empty scalar
        # queue (one table swap total); the chain's later PE stages are
        # interleaved with inactive-slot fc1 j-slices (which need no chain
        # outputs) so the PE never waits on the chain's DVE latency.
        A0 = pattern[0]
        if A0 == 0:
            st["hs"] = {}
            emit_fc1(st, st["fco"])
        else:
            emit_chains_a(st)
            if M - A0 > 0:
                cov = st["fco"][0]
                emit_fc1(st, [cov], range(0, 6))
                emit_chains_b1(st)
                emit_fc1(st, [cov], range(6, 12))
                emit_chains_b2(st)
                emit_S(st)
                emit_fc1(st, [cov], range(12, HK))
                emit_fc1(st, st["fco"][1:M - A0])
            else:
                warm2(8)
                emit_chains_b1(st)
                warm2(6)
                emit_chains_b2(st)
                emit_S(st)
            emit_fc1(st, st["fco"][M - A0:])

        for tt in range(ntt):
            nxt = None
            if tt + 1 < ntt:
                nxt = load_x(tt + 1)
            emit_u(st)
            if nxt is not None:
                emit_t1(nxt)  # adjacent to u: same tiled-cluster drain region
            emit_us_S2(st)
            g = st["fco"]
            fc2_groups = [[g[0]], [g[1]], [g[2]], [g[3]]]
            emit_fc2(st, fc2_groups[0])
            if nxt is not None:
                emit_chains_a(nxt)
            emit_fc2(st, fc2_groups[1])
            if nxt is not None:
                emit_chains_b1(nxt)
            emit_fc2(st, fc2_groups[2])
            if nxt is not None:
                emit_chains_b2(nxt)
            emit_fc2(st, fc2_groups[3])
            if nxt is not None:
                emit_S(nxt)
                emit_fc1(nxt, nxt["fco"])
                st = nxt

    nc.compile()
    return nc


def _prep_inputs(x, modality_mask, fc1_w, fc1_b, fc2_w, fc2_b, gate_w, gate_b,
                 a1_w, b1_w, a2_w, b2_w):
    """Chunk-balanced per-core input maps (numpy, host-side layout prep).

    Returns (in_maps, assign, pattern) where assign[core][slot] = (sample b,
    sample-tile tt, perm) for output scatter.
    """
    bf = lambda a: np.ascontiguousarray(a).astype(NPBF)
    f32 = lambda a: np.ascontiguousarray(a, dtype=np.float32)

    xm = np.asarray(x, np.float32).reshape(M, B, NT, C)
    # xt_all[b][tt,m,p,c*T+t] = x[m,b,tt*T+t,128c+p]
    xt_all = bf(xm.transpose(1, 0, 3, 2)            # [B,M,C,NT]
                .reshape(B, M, CK, 128, NTT, T)
                .transpose(0, 4, 1, 3, 2, 5)         # [B,NTT,M,128,CK,T]
                .reshape(B, NTT, M, 128, CK * T))

    w1h = bf(np.asarray(fc1_w, np.float32).T.reshape(CK, 128, HK, 128)
             .transpose(2, 1, 0, 3).reshape(HK, 128, CK * 128))
    w2h = bf(np.asarray(fc2_w, np.float32).T.reshape(HK, 128, C))
    a1t = np.asarray(a1_w, np.float32).transpose(2, 0, 1).reshape(CK, 128, M, 16)
    gwt = np.asarray(gate_w, np.float32).transpose(2, 0, 1).reshape(CK, 128, M, M)
    a2t = np.asarray(a2_w, np.float32).transpose(2, 0, 1).reshape(HK, 128, M, 16)
    b1t = np.asarray(b1_w, np.float32).transpose(0, 2, 1)  # [e, r, h]
    b2t = np.asarray(b2_w, np.float32).transpose(0, 2, 1)  # [e, r, c]
    gbf = np.asarray(gate_b, np.float32)
    f1bh = f32(np.asarray(fc1_b, np.float32).reshape(HK, 128).T)
    f2bh = f32(np.asarray(fc2_b, np.float32).reshape(CK, 128).T)

    maskf = np.asarray(modality_mask, np.float32)  # [M(e), B]
    # per-sample packed weights (layout independent of the compiled pattern)
    sw = []
    perms, acts = [], []
    for b in range(B):
        act = [e for e in range(M) if maskf[e, b] != 0.0]
        acts.append(act)
        perms.append(act + [e for e in range(M) if maskf[e, b] == 0.0])
        mb = maskf[:, b]
        a1p = np.zeros((CK, 128, 128), np.float32)
        a2p = np.zeros((HK, 128, 128), np.float32)
        b1p = np.zeros((128, H), np.float32)
        b2p = np.zeros((128, C), np.float32)
        gbh = np.zeros((128, 1), np.float32)
        rsm = np.zeros((128, RSW), np.float32)
        for k, e in enumerate(act):
            for j, ej in enumerate(act):
                a1p[:, :, 32 * k + j] = gwt[:, :, e, ej]   # logits for slot j
                gbh[32 * k + j, 0] = gbf[e, ej]
            a1p[:, :, 32 * k + 4:32 * k + 20] = a1t[:, :, e, :]
            a2p[:, :, 32 * k + 4:32 * k + 20] = a2t[:, :, e, :]
            b1p[32 * k + 4:32 * k + 20, :] = b1t[e]
            b2p[32 * k + 4:32 * k + 20, :] = b2t[e]
            rsm[0, 8 + 128 * k + 32 * k:8 + 128 * k + 32 * k + 4] = 1.0
            for j, ej in enumerate(act):
                rsm[32 * k + j, k] = mb[ej]
                rsm[32 * k + j, 224 + k * 128 + 32 * j + 4:
                    224 + k * 128 + 32 * j + 20] = mb[e] * mb[ej]
        sw.append(dict(
            a1=bf(a1p.transpose(1, 0, 2).reshape(128, CK * 128)),
            a2=bf(a2p.transpose(1, 0, 2).reshape(128, HK * 128)),
            b1=bf(b1p), b2=bf(b2p), gb=f32(gbh), rsm=f32(rsm)))

    # chunk balancing: sort (sample, tile) chunks by nact desc, deal
    # round-robin; compiled pattern[s] = max nact at slot s over cores
    chunks = sorted(((b, tt) for b in range(B) for tt in range(NTT)),
                    key=lambda c: (-len(acts[c[0]]), c[0], c[1]))
    ncore = B
    nslot = (len(chunks) + ncore - 1) // ncore
    assign = [[None] * nslot for _ in range(ncore)]
    for r, (b, tt) in enumerate(chunks):
        assign[r % ncore][r // ncore] = (b, tt)
    pattern = tuple(max(len(acts[assign[c][s][0]]) for c in range(ncore))
                    for s in range(nslot))

    shared = dict(w1=w1h, w2=w2h, f1b=f1bh, f2b=f2bh)
    in_maps = []
    for c in range(ncore):
        xts, a1c, a2c, b1c, b2c, gbc, rsmc = [], [], [], [], [], [], []
        for s in range(nslot):
            b, tt = assign[c][s]
            xts.append(xt_all[b][tt][perms[b]])
            w = sw[b]
            a1c.append(w["a1"]); a2c.append(w["a2"])
            b1c.append(w["b1"]); b2c.append(w["b2"])
            gbc.append(w["gb"]); rsmc.append(w["rsm"])
        in_maps.append(dict(
            shared, xt=np.ascontiguousarray(np.stack(xts)),
            a1=np.ascontiguousarray(np.stack(a1c)),
            a2=np.ascontiguousarray(np.stack(a2c)),
            b1=np.ascontiguousarray(np.stack(b1c)),
            b2=np.ascontiguousarray(np.stack(b2c)),
            gb=np.ascontiguousarray(np.stack(gbc)),
            rsm=np.ascontiguousarray(np.stack(rsmc))))
    return in_maps, assign, perms, pattern


def _run(inputs, trace=False, trace_kwargs=None):
    in_maps, assign, perms, pattern = _prep_inputs(**inputs)
    if pattern not in _CACHE:
        _CACHE[pattern] = _build_program(pattern)
    nc = _CACHE[pattern]
    kw = {}
    if trace:
        kw = dict(trace=True, trace_kwargs=trace_kwargs or {})
    res = bass_utils.run_bass_kernel_spmd(nc, in_maps, list(range(len(in_maps))), **kw)
    y = np.empty((M * B, NT, C), np.float32)
    nslot = len(pattern)
    for c in range(len(in_maps)):
        ytb = np.asarray(res.results[c]["yt"], dtype=np.float32)  # [M,nslot,2,128,3T]
        # [M, nslot, 2, 128, 3, T] -> [M, nslot, T, 2, 3, 128] -> [M,nslot,T,C]
        yb = (ytb.reshape(M, nslot, 2, 128, 3, T)
              .transpose(0, 1, 5, 2, 4, 3).reshape(M, nslot, T, C))
        for s in range(nslot):
            b, tt = assign[c][s]
            for m in range(M):
                y[perms[b][m] * B + b, tt * T:(tt + 1) * T] = yb[m, s]
    return y, res


def kernel(**inputs):
    y, _ = _run(inputs)
    return y
